# revision 33
# baseline (speedup 1.0000x reference)
"""Trainium2 Bass kernel for nn_MultiHeadSelfAttention (B=4, T=1024, DIN=512,
DLIN=1024, DK=DV=1024, NH=16).

Strategy (8 NeuronCores): core c = 2*b + g handles batch b (4 batches) and
head-group g (2 groups of 8 heads).  The input-projection matmul chain
(x -> h -> q/k/v) is folded on the host:

    q = data @ (W_q[gs] @ W_in[:, :512]).T + (W_q[gs] @ W_in[:, 512:])[:, t]

because x = [data | I_T], so the positional one-hot part of W_in is just a
per-position additive table.  This cuts device FLOPs ~2.3x and removes every
device-side transpose: the device receives pre-transposed bf16 operands and
computes, per core:

    qT, kT  [512, 1024]   (head-dim on partitions)    - folded projections
    v       [1024, 512+ones]                          - folded projection
    ST      [t2, t1] = kT^T q per head (K=64, two heads packed in the 128
                        partitions via PE row-tiling)
    P = exp(ST)          (scores are tiny: |S| < 0.6, softmax needs no max)
    attT_un [65, t1]  = [v | 1]^T P  accumulated over t2  (row 64 = denom)

Host divides by the denominator row, transposes, and assembles the full
[4, 1024, 1024] fp32 output.
"""

from contextlib import ExitStack

import numpy as np
import ml_dtypes

import concourse.bass as bass
import concourse.mybir as mybir
import concourse.tile as tile
from concourse import bacc
from concourse.bass_utils import run_bass_kernel_spmd

BF16 = mybir.dt.bfloat16
F32 = mybir.dt.float32
NPBF16 = ml_dtypes.bfloat16

B, T, DIN = 4, 1024, 512
DLIN, NH, DH = 1024, 16, 64
G = 2                # head groups (cores per batch)
HPG = NH // G        # heads per group = 8
KO = HPG * DH        # per-core projection width = 512
SCALE = 1.0 / 8.0    # 1/sqrt(dk)

_STATE = {}


def _emit(ctx: ExitStack, tc: "tile.TileContext", stage: int = 4):
    """stage: 1=input DMAs only, 2=+projections, 3=+scores/exp, 4=full."""
    nc = tc.nc
    # All inputs packed host-side into one bf16 tensor of [128, 1024] chunks:
    #   [wqt(2), datat(4), wkt(2), posqt(4), poskt(4), wvt(2), posv(4)] = 22
    inp = nc.dram_tensor("inp", [22 * 128, T], BF16, kind="ExternalInput").ap()
    out = nc.dram_tensor("attun", [HPG, DH + 1, T], F32, kind="ExternalOutput").ap()

    CT = DIN // 128   # 4 contraction tiles for the projections
    JT = KO // 128    # 4 ko-tiles (2 heads each)
    TT = T // 128     # 8 t-tiles
    H2 = T // 512     # 2 moving halves of t

    consts = ctx.enter_context(tc.tile_pool(name="consts", bufs=1))
    # ST/proj psum: [128, 1024] fp32 = 2 banks per slot, 3 slots = 6 banks.
    psum = ctx.enter_context(tc.tile_pool(name="psum", bufs=3, space="PSUM"))
    # attT psum: [65, 512] fp32 = 1 bank per slot, 2 slots.
    psum_att = ctx.enter_context(tc.tile_pool(name="psum_att", bufs=2, space="PSUM"))
    pP = ctx.enter_context(tc.tile_pool(name="pP", bufs=6))
    outp = ctx.enter_context(tc.tile_pool(name="outp", bufs=3))

    # ---- input load: one packed SBUF tile, three DMAs on separate queues in
    # the order the chunks are first needed.
    big = consts.tile([128, 22, T], BF16)
    src = inp.rearrange("(a p) t -> p a t", p=128)
    nc.sync.dma_start(out=big[:, 0:6], in_=src[:, 0:6])      # wqt, datat
    nc.scalar.dma_start(out=big[:, 8:16], in_=src[:, 8:16])  # posqt, poskt
    nc.sync.dma_start(out=big[:, 6:8], in_=src[:, 6:8])      # wkt
    nc.gpsimd.dma_start(out=big[:, 16:22], in_=src[:, 16:22])  # wvt, posv

    # piece views; a 512-wide [DIN, KO] piece packs two c-tiles per chunk:
    # chunk u holds c-tiles (2u | 2u+1) side by side.
    def wslices(base):  # -> per-c-tile [128, 512] APs
        return [big[:, base + c // 2, (c % 2) * 512:(c % 2) * 512 + 512]
                for c in range(CT)]

    wqt_c = wslices(0)
    datat_sb = big[:, 2:6]                  # [128, 4, 1024], c-tile major
    wkt_c = wslices(6)
    posqt_sb = big[:, 8:12]                 # [128, 4, 1024], ko-tile major
    poskt_sb = big[:, 12:16]
    wvt_c = wslices(16)
    posv_sb = big[:, 18:22].rearrange("p a (b k) -> p (a b) k", k=KO)  # [128, 8, 512]

    qt_sb = consts.tile([128, JT, T], BF16)
    kt_sb = consts.tile([128, JT, T], BF16)
    # v extended with a ones-column per head: [t2, 8*(64+1)]
    vext_sb = consts.tile([128, TT, HPG * (DH + 1)], BF16)
    nc.gpsimd.memset(vext_sb, 1.0)

    if stage <= 1:
        # keep outputs written so the program stays well-formed
        dummy = outp.tile([DH + 1, T], F32, name="dummy_out")
        nc.vector.memset(dummy, 0.0)
        for head in range(HPG):
            nc.sync.dma_start(out=out[head], in_=dummy)
        return

    # ---- projection emitters (PE matmul + DVE pos-add evacuation)
    def emit_projqk(j):
        for (w_c, pos_sb, dst) in ((wqt_c, posqt_sb, qt_sb), (wkt_c, poskt_sb, kt_sb)):
            ps = psum.tile([128, T], F32, tag="st", name="ps_proj")
            for h2 in range(H2):
                for c in range(CT):
                    nc.tensor.matmul(
                        ps[:, h2 * 512:(h2 + 1) * 512],
                        lhsT=w_c[c][:, j * 128:(j + 1) * 128],
                        rhs=datat_sb[:, c, h2 * 512:(h2 + 1) * 512],
                        start=(c == 0),
                        stop=(c == CT - 1),
                    )
            nc.vector.tensor_add(dst[:, j, :], ps, pos_sb[:, j, :])

    def emit_projv():
        # v[t, ko] = sum_c datat[c, t] * wvt[c, ko] (+ pos), written into the
        # per-head strided vext layout (64 cols out of 65).
        for i in range(TT):
            ps = psum.tile([128, 512], F32, tag="st", name="ps_v")
            for c in range(CT):
                nc.tensor.matmul(
                    ps,
                    lhsT=datat_sb[:, c, i * 128:(i + 1) * 128],
                    rhs=wvt_c[c],
                    start=(c == 0),
                    stop=(c == CT - 1),
                )
            dst = vext_sb[:, i].rearrange("p (h x) -> p h x", x=DH + 1)[:, :, 0:DH]
            nc.vector.tensor_add(
                dst,
                ps.rearrange("p (h x) -> p h x", x=DH),
                posv_sb[:, i].rearrange("p (h x) -> p h x", x=DH),
            )

    def emit_st(j, p_tiles):
        # scores (transposed): ST[t2, t1] = sum_ko kT[ko, t2] qT[ko, t1].
        # K=64; the two heads of the pair occupy partition strips [0:64) and
        # [64:128) -> PE row-tiling runs their matmuls concurrently.
        for tt in range(TT):
            for hb in range(2):
                ks = kt_sb[hb * 64:(hb + 1) * 64]
                qs = qt_sb[hb * 64:(hb + 1) * 64]
                ps = psum.tile([128, T], F32, tag="st", name="ps_st")
                for h2 in range(H2):
                    nc.tensor.matmul(
                        ps[:, h2 * 512:(h2 + 1) * 512],
                        lhsT=ks[:, j, tt * 128:(tt + 1) * 128],
                        rhs=qs[:, j, h2 * 512:(h2 + 1) * 512],
                        start=True,
                        stop=True,
                    )
                # exp straight out of PSUM (FD=1024 per ACT instruction)
                nc.scalar.activation(
                    p_tiles[hb][:, tt, :],
                    ps,
                    mybir.ActivationFunctionType.Exp,
                )

    def emit_att(j, p_tiles):
        # attT_un[dv+1, t1] = sum_t2 [v|1][t2, dv+1] * P[t2, t1]
        for hb in range(2):
            head = 2 * j + hb
            out_sb = outp.tile([DH + 1, T], F32, name="att_out")
            for h2 in range(H2):
                pa = psum_att.tile([DH + 1, 512], F32, tag="att", name="ps_att")
                for tt in range(TT):
                    nc.tensor.matmul(
                        pa,
                        lhsT=vext_sb[:, tt, head * (DH + 1):(head + 1) * (DH + 1)],
                        rhs=p_tiles[hb][:, tt, h2 * 512:(h2 + 1) * 512],
                        start=(tt == 0),
                        stop=(tt == TT - 1),
                    )
                nc.vector.tensor_copy(out_sb[:, h2 * 512:(h2 + 1) * 512], pa)
            nc.gpsimd.dma_start(out=out[head], in_=out_sb)

    # ---- emission: pull the first score/exp stream as early as its deps
    # allow (ACT is the bottleneck engine), pipeline the previous pair's attT
    # into each subsequent pair's ACT-bound stretch.
    if stage == 2:
        for j in range(JT):
            emit_projqk(j)
        emit_projv()
    if stage >= 3:
        def ptiles(j):
            return [pP.tile([128, TT, T], BF16, tag="P", name=f"p_{j}_{hb}")
                    for hb in range(2)]
        emit_projqk(0)
        emit_projqk(1)
        p0 = ptiles(0)
        emit_st(0, p0)
        emit_projqk(2)
        emit_projv()
        p1 = ptiles(1)
        emit_st(1, p1)
        if stage >= 4:
            emit_att(0, p0)
        emit_projqk(3)
        p2 = ptiles(2)
        emit_st(2, p2)
        if stage >= 4:
            emit_att(1, p1)
        p3 = ptiles(3)
        emit_st(3, p3)
        if stage >= 4:
            emit_att(2, p2)
            emit_att(3, p3)

    if stage <= 3:
        dummy = outp.tile([DH + 1, T], F32, name="dummy_out")
        nc.vector.memset(dummy, 0.0)
        for head in range(HPG):
            nc.sync.dma_start(out=out[head], in_=dummy)


def _build_nc(repeat: int = 1, stage: int = 4):
    """repeat > 1 wraps the body in a device-side loop (for benchmarking)."""
    nc = bacc.Bacc()
    with tile.TileContext(nc) as tc:
        with ExitStack() as ctx:
            if repeat == 1:
                _emit(ctx, tc, stage)
            else:
                with tc.For_i(0, repeat, 1,
                              hint_engines=(mybir.EngineType.PE,
                                            mybir.EngineType.Activation)):
                    _emit(ctx, tc, stage)
    nc.compile()
    return nc


def _get_nc():
    if "nc" not in _STATE:
        _STATE["nc"] = _build_nc()
    return _STATE["nc"]


def _chunks(a):
    """[rows, 512|1024] fp -> [n, 128, 1024] chunk array (pairs packed)."""
    t = a.reshape(-1, 128, a.shape[1])
    if a.shape[1] == 512:
        t = np.concatenate([t[0::2], t[1::2]], axis=2)
    return t


def _prep_inputs(data, W_in, W_q, W_k, W_v):
    """Host-side weight folding + sharding. Returns per-core input maps."""
    w_in_d = W_in[:, :DIN]          # data part  [DLIN, DIN]
    w_in_p = W_in[:, DIN:]          # positional [DLIN, T]
    per_g = []
    for g in range(G):
        gs = slice(KO * g, KO * (g + 1))
        per_g.append({
            "wqt": ((W_q[gs] @ w_in_d) * SCALE).T,
            "wkt": (W_k[gs] @ w_in_d).T,
            "wvt": (W_v[gs] @ w_in_d).T,
            "posqt": (W_q[gs] @ w_in_p) * SCALE,
            "poskt": W_k[gs] @ w_in_p,
            "posv": (W_v[gs] @ w_in_p).T,
        })
    in_maps = []
    for b in range(B):
        dt_b = data[b].T
        for g in range(G):
            p = per_g[g]
            packed = np.concatenate([
                _chunks(p["wqt"]), _chunks(dt_b), _chunks(p["wkt"]),
                _chunks(p["posqt"]), _chunks(p["poskt"]),
                _chunks(p["wvt"]), _chunks(p["posv"]),
            ]).astype(NPBF16).reshape(22 * 128, T)
            in_maps.append({"inp": packed})
    return in_maps


def _assemble(results):
    """Divide by denominators, transpose, and pack the full output."""
    out = np.empty((B, T, NH * DH), dtype=np.float32)
    for core, res in enumerate(results):
        b, g = divmod(core, G)
        att_un = res["attun"]                      # [8, 65, 1024]
        att = att_un[:, :DH, :] / att_un[:, DH:DH + 1, :]
        # att: [8 heads, 64 dv, 1024 t] -> out cols [512g + 64h + dv]
        blk = att.transpose(2, 0, 1).reshape(T, KO)
        out[b, :, KO * g:KO * (g + 1)] = blk
    return out


def kernel(**inputs):
    data = np.asarray(inputs["data"], dtype=np.float32)
    W_in = np.asarray(inputs["W_in"], dtype=np.float32)
    W_q = np.asarray(inputs["W_q"], dtype=np.float32)
    W_k = np.asarray(inputs["W_k"], dtype=np.float32)
    W_v = np.asarray(inputs["W_v"], dtype=np.float32)

    in_maps = _prep_inputs(data, W_in, W_q, W_k, W_v)
    nc = _get_nc()
    res = run_bass_kernel_spmd(nc, in_maps, core_ids=list(range(B * G)))
    return _assemble(res.results)


# revision 34
# speedup vs baseline: 1.1296x; 1.1296x over previous
"""Trainium2 Bass kernel for nn_MultiHeadSelfAttention (B=4, T=1024, DIN=512,
DLIN=1024, DK=DV=1024, NH=16).

Strategy (8 NeuronCores): core c = 2*b + g handles batch b (4 batches) and
head-group g (2 groups of 8 heads).  The input-projection matmul chain
(x -> h -> q/k/v) is folded on the host:

    q = data @ (W_q[gs] @ W_in[:, :512]).T + (W_q[gs] @ W_in[:, 512:])[:, t]

because x = [data | I_T], so the positional one-hot part of W_in is just a
per-position additive table.  This cuts device FLOPs ~2.3x and removes every
device-side transpose: the device receives pre-transposed bf16 operands and
computes, per core:

    qT, kT  [512, 1024]   (head-dim on partitions)    - folded projections
    v       [1024, 512+ones]                          - folded projection
    ST      [t2, t1] = kT^T q per head (K=64, two heads packed in the 128
                        partitions via PE row-tiling)
    P = exp(ST)          (scores are tiny: |S| < 0.6, softmax needs no max)
    attT_un [65, t1]  = [v | 1]^T P  accumulated over t2  (row 64 = denom)

Host divides by the denominator row, transposes, and assembles the full
[4, 1024, 1024] fp32 output.
"""

from contextlib import ExitStack

import numpy as np
import ml_dtypes

import concourse.bass as bass
import concourse.mybir as mybir
import concourse.tile as tile
from concourse import bacc
from concourse.bass_utils import run_bass_kernel_spmd

BF16 = mybir.dt.bfloat16
F32 = mybir.dt.float32
NPBF16 = ml_dtypes.bfloat16

B, T, DIN = 4, 1024, 512
DLIN, NH, DH = 1024, 16, 64
G = 2                # head groups (cores per batch)
HPG = NH // G        # heads per group = 8
KO = HPG * DH        # per-core projection width = 512
SCALE = 1.0 / 8.0    # 1/sqrt(dk)

_STATE = {}


def _emit(ctx: ExitStack, tc: "tile.TileContext", stage: int = 4):
    """stage: 1=input DMAs only, 2=+projections, 3=+scores/exp, 4=full."""
    nc = tc.nc
    # All inputs packed host-side into one bf16 tensor of [128, 1024] chunks:
    #   [wqt(2), datat(4), wkt(2), posqt(4), poskt(4), wvt(2), posv(4)] = 22
    inp = nc.dram_tensor("inp", [22 * 128, T], BF16, kind="ExternalInput").ap()
    out = nc.dram_tensor("attun", [HPG, DH + 1, T], F32, kind="ExternalOutput").ap()

    CT = DIN // 128   # 4 contraction tiles for the projections
    JT = KO // 128    # 4 ko-tiles (2 heads each)
    TT = T // 128     # 8 t-tiles
    H2 = T // 512     # 2 moving halves of t

    consts = ctx.enter_context(tc.tile_pool(name="consts", bufs=1))
    # ST/proj psum: [128, 1024] fp32 = 2 banks per slot, 3 slots = 6 banks.
    psum = ctx.enter_context(tc.tile_pool(name="psum", bufs=3, space="PSUM"))
    # attT psum: [65, 512] fp32 = 1 bank per slot, 2 slots.
    psum_att = ctx.enter_context(tc.tile_pool(name="psum_att", bufs=2, space="PSUM"))
    pP = ctx.enter_context(tc.tile_pool(name="pP", bufs=6))
    outp = ctx.enter_context(tc.tile_pool(name="outp", bufs=3))

    # ---- input load: one packed SBUF tile, three DMAs on separate queues in
    # the order the chunks are first needed.
    big = consts.tile([128, 22, T], BF16)
    src = inp.rearrange("(a p) t -> p a t", p=128)
    nc.sync.dma_start(out=big[:, 0:6], in_=src[:, 0:6])      # wqt, datat
    nc.scalar.dma_start(out=big[:, 8:16], in_=src[:, 8:16])  # posqt, poskt
    nc.sync.dma_start(out=big[:, 6:8], in_=src[:, 6:8])      # wkt
    nc.gpsimd.dma_start(out=big[:, 16:22], in_=src[:, 16:22])  # wvt, posv

    # piece views; a 512-wide [DIN, KO] piece packs two c-tiles per chunk:
    # chunk u holds c-tiles (2u | 2u+1) side by side.
    def wslices(base):  # -> per-c-tile [128, 512] APs
        return [big[:, base + c // 2, (c % 2) * 512:(c % 2) * 512 + 512]
                for c in range(CT)]

    wqt_c = wslices(0)
    datat_sb = big[:, 2:6]                  # [128, 4, 1024], c-tile major
    wkt_c = wslices(6)
    posqt_sb = big[:, 8:12]                 # [128, 4, 1024], ko-tile major
    poskt_sb = big[:, 12:16]
    wvt_c = wslices(16)
    posv_sb = big[:, 18:22].rearrange("p a (b k) -> p (a b) k", k=KO)  # [128, 8, 512]

    qt_sb = consts.tile([128, JT, T], BF16)
    kt_sb = consts.tile([128, JT, T], BF16)
    # v extended with a ones-column per head: [t2, 8*(64+1)]
    vext_sb = consts.tile([128, TT, HPG * (DH + 1)], BF16)
    nc.gpsimd.memset(vext_sb, 1.0)

    if stage <= 1:
        # keep outputs written so the program stays well-formed
        dummy = outp.tile([DH + 1, T], F32, name="dummy_out")
        nc.vector.memset(dummy, 0.0)
        for head in range(HPG):
            nc.sync.dma_start(out=out[head], in_=dummy)
        return

    # ---- projection emitters (PE matmul + DVE pos-add evacuation)
    def emit_proj_one(j, which):
        w_c, pos_sb, dst = ((wqt_c, posqt_sb, qt_sb),
                            (wkt_c, poskt_sb, kt_sb))[which]
        ps = psum.tile([128, T], F32, tag="st", name="ps_proj")
        for h2 in range(H2):
            for c in range(CT):
                nc.tensor.matmul(
                    ps[:, h2 * 512:(h2 + 1) * 512],
                    lhsT=w_c[c][:, j * 128:(j + 1) * 128],
                    rhs=datat_sb[:, c, h2 * 512:(h2 + 1) * 512],
                    start=(c == 0),
                    stop=(c == CT - 1),
                )
        nc.vector.tensor_add(dst[:, j, :], ps, pos_sb[:, j, :])

    def emit_projqk(j):
        emit_proj_one(j, 0)
        emit_proj_one(j, 1)

    def emit_projv_one(i):
        # v[t, ko] = sum_c datat[c, t] * wvt[c, ko] (+ pos), written into the
        # per-head strided vext layout (64 cols out of 65).
        if True:
            ps = psum.tile([128, 512], F32, tag="st", name="ps_v")
            for c in range(CT):
                nc.tensor.matmul(
                    ps,
                    lhsT=datat_sb[:, c, i * 128:(i + 1) * 128],
                    rhs=wvt_c[c],
                    start=(c == 0),
                    stop=(c == CT - 1),
                )
            dst = vext_sb[:, i].rearrange("p (h x) -> p h x", x=DH + 1)[:, :, 0:DH]
            nc.vector.tensor_add(
                dst,
                ps.rearrange("p (h x) -> p h x", x=DH),
                posv_sb[:, i].rearrange("p (h x) -> p h x", x=DH),
            )

    def emit_st(j, p_tiles, fillers=None):
        # scores (transposed): ST[t2, t1] = sum_ko kT[ko, t2] qT[ko, t1].
        # K=64; the two heads of the pair occupy partition strips [0:64) and
        # [64:128) -> PE row-tiling runs their matmuls concurrently.  Filler
        # units (~1.7us of other PE work each) are injected between score
        # tiles so the ACT exp stream never starves behind a long PE block.
        for tt in range(TT):
            for hb in range(2):
                if fillers:
                    fillers.popleft()()
                ks = kt_sb[hb * 64:(hb + 1) * 64]
                qs = qt_sb[hb * 64:(hb + 1) * 64]
                ps = psum.tile([128, T], F32, tag="st", name="ps_st")
                for h2 in range(H2):
                    nc.tensor.matmul(
                        ps[:, h2 * 512:(h2 + 1) * 512],
                        lhsT=ks[:, j, tt * 128:(tt + 1) * 128],
                        rhs=qs[:, j, h2 * 512:(h2 + 1) * 512],
                        start=True,
                        stop=True,
                    )
                # exp straight out of PSUM (FD=1024 per ACT instruction)
                nc.scalar.activation(
                    p_tiles[hb][:, tt, :],
                    ps,
                    mybir.ActivationFunctionType.Exp,
                )

    def emit_att_unit(j, p_tiles, hb, h2, out_sb):
        # one attT accumulation group: attT_un[dv+1, t1-half] for one head
        head = 2 * j + hb
        pa = psum_att.tile([DH + 1, 512], F32, tag="att", name="ps_att")
        for tt in range(TT):
            nc.tensor.matmul(
                pa,
                lhsT=vext_sb[:, tt, head * (DH + 1):(head + 1) * (DH + 1)],
                rhs=p_tiles[hb][:, tt, h2 * 512:(h2 + 1) * 512],
                start=(tt == 0),
                stop=(tt == TT - 1),
            )
        nc.vector.tensor_copy(out_sb[:, h2 * 512:(h2 + 1) * 512], pa)
        if h2 == H2 - 1:
            nc.gpsimd.dma_start(out=out[head], in_=out_sb)

    def att_units(j, p_tiles):
        units = []
        for hb in range(2):
            out_sb = outp.tile([DH + 1, T], F32, name="att_out")
            for h2 in range(H2):
                units.append(lambda hb=hb, h2=h2, sb=out_sb:
                             emit_att_unit(j, p_tiles, hb, h2, sb))
        return units

    def emit_att(j, p_tiles):
        for u in att_units(j, p_tiles):
            u()

    # ---- emission: pull the first score/exp stream as early as its deps
    # allow (ACT is the bottleneck engine), pipeline the previous pair's attT
    # into each subsequent pair's ACT-bound stretch.
    if stage == 2:
        for j in range(JT):
            emit_projqk(j)
        emit_projv()
    if stage >= 3:
        def ptiles(j):
            return [pP.tile([128, TT, T], BF16, tag="P", name=f"p_{j}_{hb}")
                    for hb in range(2)]
        from collections import deque
        emit_projqk(0)
        emit_projqk(1)
        p0 = ptiles(0)
        fill = deque()
        for j in (2, 3):
            for which in (0, 1):
                fill.append(lambda j=j, w=which: emit_proj_one(j, w))
        for i in range(TT):
            fill.append(lambda i=i: emit_projv_one(i))
        emit_st(0, p0, fill)
        while fill:
            fill.popleft()()
        p1 = ptiles(1)
        fill = deque(att_units(0, p0)) if stage >= 4 else deque()
        emit_st(1, p1, fill)
        while fill:
            fill.popleft()()
        p2 = ptiles(2)
        fill = deque(att_units(1, p1)) if stage >= 4 else deque()
        emit_st(2, p2, fill)
        while fill:
            fill.popleft()()
        p3 = ptiles(3)
        fill = deque(att_units(2, p2)) if stage >= 4 else deque()
        emit_st(3, p3, fill)
        while fill:
            fill.popleft()()
        if stage >= 4:
            emit_att(3, p3)

    if stage <= 3:
        dummy = outp.tile([DH + 1, T], F32, name="dummy_out")
        nc.vector.memset(dummy, 0.0)
        for head in range(HPG):
            nc.sync.dma_start(out=out[head], in_=dummy)


def _build_nc(repeat: int = 1, stage: int = 4):
    """repeat > 1 wraps the body in a device-side loop (for benchmarking)."""
    nc = bacc.Bacc()
    with tile.TileContext(nc) as tc:
        with ExitStack() as ctx:
            if repeat == 1:
                _emit(ctx, tc, stage)
            else:
                with tc.For_i(0, repeat, 1,
                              hint_engines=(mybir.EngineType.PE,
                                            mybir.EngineType.Activation)):
                    _emit(ctx, tc, stage)
    nc.compile()
    return nc


def _get_nc():
    if "nc" not in _STATE:
        _STATE["nc"] = _build_nc()
    return _STATE["nc"]


def _chunks(a):
    """[rows, 512|1024] fp -> [n, 128, 1024] chunk array (pairs packed)."""
    t = a.reshape(-1, 128, a.shape[1])
    if a.shape[1] == 512:
        t = np.concatenate([t[0::2], t[1::2]], axis=2)
    return t


def _prep_inputs(data, W_in, W_q, W_k, W_v):
    """Host-side weight folding + sharding. Returns per-core input maps."""
    w_in_d = W_in[:, :DIN]          # data part  [DLIN, DIN]
    w_in_p = W_in[:, DIN:]          # positional [DLIN, T]
    per_g = []
    for g in range(G):
        gs = slice(KO * g, KO * (g + 1))
        per_g.append({
            "wqt": ((W_q[gs] @ w_in_d) * SCALE).T,
            "wkt": (W_k[gs] @ w_in_d).T,
            "wvt": (W_v[gs] @ w_in_d).T,
            "posqt": (W_q[gs] @ w_in_p) * SCALE,
            "poskt": W_k[gs] @ w_in_p,
            "posv": (W_v[gs] @ w_in_p).T,
        })
    in_maps = []
    for b in range(B):
        dt_b = data[b].T
        for g in range(G):
            p = per_g[g]
            packed = np.concatenate([
                _chunks(p["wqt"]), _chunks(dt_b), _chunks(p["wkt"]),
                _chunks(p["posqt"]), _chunks(p["poskt"]),
                _chunks(p["wvt"]), _chunks(p["posv"]),
            ]).astype(NPBF16).reshape(22 * 128, T)
            in_maps.append({"inp": packed})
    return in_maps


def _assemble(results):
    """Divide by denominators, transpose, and pack the full output."""
    out = np.empty((B, T, NH * DH), dtype=np.float32)
    for core, res in enumerate(results):
        b, g = divmod(core, G)
        att_un = res["attun"]                      # [8, 65, 1024]
        att = att_un[:, :DH, :] / att_un[:, DH:DH + 1, :]
        # att: [8 heads, 64 dv, 1024 t] -> out cols [512g + 64h + dv]
        blk = att.transpose(2, 0, 1).reshape(T, KO)
        out[b, :, KO * g:KO * (g + 1)] = blk
    return out


def kernel(**inputs):
    data = np.asarray(inputs["data"], dtype=np.float32)
    W_in = np.asarray(inputs["W_in"], dtype=np.float32)
    W_q = np.asarray(inputs["W_q"], dtype=np.float32)
    W_k = np.asarray(inputs["W_k"], dtype=np.float32)
    W_v = np.asarray(inputs["W_v"], dtype=np.float32)

    in_maps = _prep_inputs(data, W_in, W_q, W_k, W_v)
    nc = _get_nc()
    res = run_bass_kernel_spmd(nc, in_maps, core_ids=list(range(B * G)))
    return _assemble(res.results)


# revision 36
# speedup vs baseline: 1.1916x; 1.0549x over previous
"""Trainium2 Bass kernel for nn_MultiHeadSelfAttention (B=4, T=1024, DIN=512,
DLIN=1024, DK=DV=1024, NH=16).

Strategy (8 NeuronCores): core c = 2*b + g handles batch b (4 batches) and
head-group g (2 groups of 8 heads).  The input-projection matmul chain
(x -> h -> q/k/v) is folded on the host:

    q = data @ (W_q[gs] @ W_in[:, :512]).T + (W_q[gs] @ W_in[:, 512:])[:, t]

because x = [data | I_T], so the positional one-hot part of W_in is just a
per-position additive table.  This cuts device FLOPs ~2.3x and removes every
device-side transpose: the device receives pre-transposed bf16 operands and
computes, per core:

    qT, kT  [512, 1024]   (head-dim on partitions)    - folded projections
    v       [1024, 512+ones]                          - folded projection
    ST      [t2, t1] = kT^T q per head (K=64, two heads packed in the 128
                        partitions via PE row-tiling)
    P = exp(ST)          (scores are tiny: |S| < 0.6, softmax needs no max)
    attT_un [65, t1]  = [v | 1]^T P  accumulated over t2  (row 64 = denom)

Host divides by the denominator row, transposes, and assembles the full
[4, 1024, 1024] fp32 output.
"""

from contextlib import ExitStack

import numpy as np
import ml_dtypes

import concourse.bass as bass
import concourse.mybir as mybir
import concourse.tile as tile
from concourse import bacc
from concourse.bass_utils import run_bass_kernel_spmd

BF16 = mybir.dt.bfloat16
F32 = mybir.dt.float32
NPBF16 = ml_dtypes.bfloat16

B, T, DIN = 4, 1024, 512
DLIN, NH, DH = 1024, 16, 64
G = 2                # head groups (cores per batch)
HPG = NH // G        # heads per group = 8
KO = HPG * DH        # per-core projection width = 512
SCALE = 1.0 / 8.0    # 1/sqrt(dk)

_STATE = {}


def _emit(ctx: ExitStack, tc: "tile.TileContext", stage: int = 4):
    """stage: 1=input DMAs only, 2=+projections, 3=+scores/exp, 4=full."""
    nc = tc.nc
    # All inputs packed host-side into one bf16 tensor of [128, 1024] chunks:
    #   [wqt(2), datat(4), wkt(2), posqt(4), poskt(4), wvt(2), posv(4)] = 22
    inp = nc.dram_tensor("inp", [22 * 128, T], BF16, kind="ExternalInput").ap()
    out = nc.dram_tensor("attun", [HPG, DH + 1, T], F32, kind="ExternalOutput").ap()

    CT = DIN // 128   # 4 contraction tiles for the projections
    JT = KO // 128    # 4 ko-tiles (2 heads each)
    TT = T // 128     # 8 t-tiles
    H2 = T // 512     # 2 moving halves of t

    consts = ctx.enter_context(tc.tile_pool(name="consts", bufs=1))
    # ST/proj psum: [128, 1024] fp32 = 2 banks per slot, 3 slots = 6 banks.
    psum = ctx.enter_context(tc.tile_pool(name="psum", bufs=3, space="PSUM"))
    # attT psum: [65, 512] fp32 = 1 bank per slot, 2 slots.
    psum_att = ctx.enter_context(tc.tile_pool(name="psum_att", bufs=2, space="PSUM"))
    pP = ctx.enter_context(tc.tile_pool(name="pP", bufs=6))
    outp = ctx.enter_context(tc.tile_pool(name="outp", bufs=3))

    # ---- input load: one packed SBUF tile, three DMAs on separate queues in
    # the order the chunks are first needed.
    big = consts.tile([128, 22, T], BF16)
    src = inp.rearrange("(a p) t -> p a t", p=128)
    nc.sync.dma_start(out=big[:, 0:6], in_=src[:, 0:6])      # wqt, datat
    # pos chunks are interleaved host-side (pq0,pk0,pq1,pk1,...); the first
    # split delivers the j=0 tables early so the first evacs (and therefore
    # the exp stream) start ~4us sooner.
    nc.scalar.dma_start(out=big[:, 8:10], in_=src[:, 8:10])
    nc.scalar.dma_start(out=big[:, 10:16], in_=src[:, 10:16])
    nc.sync.dma_start(out=big[:, 6:8], in_=src[:, 6:8])      # wkt
    nc.sync.dma_start(out=big[:, 16:22], in_=src[:, 16:22])  # wvt, posv

    # piece views; a 512-wide [DIN, KO] piece packs two c-tiles per chunk:
    # chunk u holds c-tiles (2u | 2u+1) side by side.
    def wslices(base):  # -> per-c-tile [128, 512] APs
        return [big[:, base + c // 2, (c % 2) * 512:(c % 2) * 512 + 512]
                for c in range(CT)]

    wqt_c = wslices(0)
    datat_sb = big[:, 2:6]                  # [128, 4, 1024], c-tile major
    wkt_c = wslices(6)
    pos_qk = big[:, 8:16].rearrange("p (a b) t -> p a b t", b=2)
    posqt_sb = pos_qk[:, :, 0]              # [128, 4, 1024], ko-tile major
    poskt_sb = pos_qk[:, :, 1]
    wvt_c = wslices(16)
    posv_sb = big[:, 18:22].rearrange("p a (b k) -> p (a b) k", k=KO)  # [128, 8, 512]

    qt_sb = consts.tile([128, JT, T], BF16)
    kt_sb = consts.tile([128, JT, T], BF16)
    # v extended with a ones-column per head: [t2, 8*(64+1)]
    vext_sb = consts.tile([128, TT, HPG * (DH + 1)], BF16)
    nc.gpsimd.memset(vext_sb, 1.0)

    if stage <= 1:
        # keep outputs written so the program stays well-formed
        dummy = outp.tile([DH + 1, T], F32, name="dummy_out")
        nc.vector.memset(dummy, 0.0)
        for head in range(HPG):
            nc.sync.dma_start(out=out[head], in_=dummy)
        return

    # ---- projection emitters (PE matmul + DVE pos-add evacuation)
    def emit_proj_one(j, which):
        w_c, pos_sb, dst = ((wqt_c, posqt_sb, qt_sb),
                            (wkt_c, poskt_sb, kt_sb))[which]
        ps = psum.tile([128, T], F32, tag="st", name="ps_proj")
        for h2 in range(H2):
            for c in range(CT):
                nc.tensor.matmul(
                    ps[:, h2 * 512:(h2 + 1) * 512],
                    lhsT=w_c[c][:, j * 128:(j + 1) * 128],
                    rhs=datat_sb[:, c, h2 * 512:(h2 + 1) * 512],
                    start=(c == 0),
                    stop=(c == CT - 1),
                )
        nc.vector.tensor_add(dst[:, j, :], ps, pos_sb[:, j, :])

    def emit_projqk(j):
        emit_proj_one(j, 0)
        emit_proj_one(j, 1)

    def emit_projv_one(i):
        # v[t, ko] = sum_c datat[c, t] * wvt[c, ko] (+ pos), written into the
        # per-head strided vext layout (64 cols out of 65).
        if True:
            ps = psum.tile([128, 512], F32, tag="st", name="ps_v")
            for c in range(CT):
                nc.tensor.matmul(
                    ps,
                    lhsT=datat_sb[:, c, i * 128:(i + 1) * 128],
                    rhs=wvt_c[c],
                    start=(c == 0),
                    stop=(c == CT - 1),
                )
            dst = vext_sb[:, i].rearrange("p (h x) -> p h x", x=DH + 1)[:, :, 0:DH]
            nc.vector.tensor_add(
                dst,
                ps.rearrange("p (h x) -> p h x", x=DH),
                posv_sb[:, i].rearrange("p (h x) -> p h x", x=DH),
            )

    def emit_st(j, p_tiles, fillers=None):
        # scores (transposed): ST[t2, t1] = sum_ko kT[ko, t2] qT[ko, t1].
        # K=64; the two heads of the pair occupy partition strips [0:64) and
        # [64:128) -> PE row-tiling runs their matmuls concurrently.  Filler
        # units (~1.7us of other PE work each) are injected between score
        # tiles so the ACT exp stream never starves behind a long PE block.
        for tt in range(TT):
            for hb in range(2):
                if fillers:
                    fillers.popleft()()
                ks = kt_sb[hb * 64:(hb + 1) * 64]
                qs = qt_sb[hb * 64:(hb + 1) * 64]
                ps = psum.tile([128, T], F32, tag="st", name="ps_st")
                for h2 in range(H2):
                    nc.tensor.matmul(
                        ps[:, h2 * 512:(h2 + 1) * 512],
                        lhsT=ks[:, j, tt * 128:(tt + 1) * 128],
                        rhs=qs[:, j, h2 * 512:(h2 + 1) * 512],
                        start=True,
                        stop=True,
                    )
                # exp straight out of PSUM (FD=1024 per ACT instruction)
                nc.scalar.activation(
                    p_tiles[hb][:, tt, :],
                    ps,
                    mybir.ActivationFunctionType.Exp,
                )

    def emit_att_unit(j, p_tiles, hb, h2, out_sb):
        # one attT accumulation group: attT_un[dv+1, t1-half] for one head
        head = 2 * j + hb
        pa = psum_att.tile([DH + 1, 512], F32, tag="att", name="ps_att")
        for tt in range(TT):
            nc.tensor.matmul(
                pa,
                lhsT=vext_sb[:, tt, head * (DH + 1):(head + 1) * (DH + 1)],
                rhs=p_tiles[hb][:, tt, h2 * 512:(h2 + 1) * 512],
                start=(tt == 0),
                stop=(tt == TT - 1),
            )
        nc.vector.tensor_copy(out_sb[:, h2 * 512:(h2 + 1) * 512], pa)
        if h2 == H2 - 1:
            nc.gpsimd.dma_start(out=out[head], in_=out_sb)

    def att_units(j, p_tiles):
        units = []
        for hb in range(2):
            out_sb = outp.tile([DH + 1, T], F32, name="att_out")
            for h2 in range(H2):
                units.append(lambda hb=hb, h2=h2, sb=out_sb:
                             emit_att_unit(j, p_tiles, hb, h2, sb))
        return units

    def emit_att(j, p_tiles):
        for u in att_units(j, p_tiles):
            u()

    # ---- emission: pull the first score/exp stream as early as its deps
    # allow (ACT is the bottleneck engine), pipeline the previous pair's attT
    # into each subsequent pair's ACT-bound stretch.
    if stage == 2:
        for j in range(JT):
            emit_projqk(j)
        emit_projv()
    if stage >= 3:
        def ptiles(j):
            return [pP.tile([128, TT, T], BF16, tag="P", name=f"p_{j}_{hb}")
                    for hb in range(2)]
        from collections import deque
        emit_projqk(0)
        emit_projqk(1)
        p0 = ptiles(0)
        fill = deque()
        for j in (2, 3):
            for which in (0, 1):
                fill.append(lambda j=j, w=which: emit_proj_one(j, w))
        for i in range(TT):
            fill.append(lambda i=i: emit_projv_one(i))
        emit_st(0, p0, fill)
        while fill:
            fill.popleft()()
        p1 = ptiles(1)
        fill = deque(att_units(0, p0)) if stage >= 4 else deque()
        emit_st(1, p1, fill)
        while fill:
            fill.popleft()()
        p2 = ptiles(2)
        fill = deque(att_units(1, p1)) if stage >= 4 else deque()
        emit_st(2, p2, fill)
        while fill:
            fill.popleft()()
        p3 = ptiles(3)
        fill = deque(att_units(2, p2)) if stage >= 4 else deque()
        emit_st(3, p3, fill)
        while fill:
            fill.popleft()()
        if stage >= 4:
            emit_att(3, p3)

    if stage <= 3:
        dummy = outp.tile([DH + 1, T], F32, name="dummy_out")
        nc.vector.memset(dummy, 0.0)
        for head in range(HPG):
            nc.sync.dma_start(out=out[head], in_=dummy)


def _build_nc(repeat: int = 1, stage: int = 4):
    """repeat > 1 wraps the body in a device-side loop (for benchmarking)."""
    nc = bacc.Bacc()
    with tile.TileContext(nc) as tc:
        with ExitStack() as ctx:
            if repeat == 1:
                _emit(ctx, tc, stage)
            else:
                with tc.For_i(0, repeat, 1,
                              hint_engines=(mybir.EngineType.PE,
                                            mybir.EngineType.Activation)):
                    _emit(ctx, tc, stage)
    nc.compile()
    return nc


def _get_nc():
    if "nc" not in _STATE:
        _STATE["nc"] = _build_nc()
    return _STATE["nc"]


def _chunks(a):
    """[rows, 512|1024] fp -> [n, 128, 1024] chunk array (pairs packed)."""
    t = a.reshape(-1, 128, a.shape[1])
    if a.shape[1] == 512:
        t = np.concatenate([t[0::2], t[1::2]], axis=2)
    return t


def _prep_inputs(data, W_in, W_q, W_k, W_v):
    """Host-side weight folding + sharding. Returns per-core input maps."""
    w_in_d = W_in[:, :DIN]          # data part  [DLIN, DIN]
    w_in_p = W_in[:, DIN:]          # positional [DLIN, T]
    per_g = []
    for g in range(G):
        gs = slice(KO * g, KO * (g + 1))
        per_g.append({
            "wqt": ((W_q[gs] @ w_in_d) * SCALE).T,
            "wkt": (W_k[gs] @ w_in_d).T,
            "wvt": (W_v[gs] @ w_in_d).T,
            "posqt": (W_q[gs] @ w_in_p) * SCALE,
            "poskt": W_k[gs] @ w_in_p,
            "posv": (W_v[gs] @ w_in_p).T,
        })
    in_maps = []
    for b in range(B):
        dt_b = data[b].T
        for g in range(G):
            p = per_g[g]
            pos = np.empty((8, 128, T), dtype=p["posqt"].dtype)
            pos[0::2] = _chunks(p["posqt"])
            pos[1::2] = _chunks(p["poskt"])
            packed = np.concatenate([
                _chunks(p["wqt"]), _chunks(dt_b), _chunks(p["wkt"]),
                pos, _chunks(p["wvt"]), _chunks(p["posv"]),
            ]).astype(NPBF16).reshape(22 * 128, T)
            in_maps.append({"inp": packed})
    return in_maps


def _assemble(results):
    """Divide by denominators, transpose, and pack the full output."""
    out = np.empty((B, T, NH * DH), dtype=np.float32)
    for core, res in enumerate(results):
        b, g = divmod(core, G)
        att_un = res["attun"]                      # [8, 65, 1024]
        att = att_un[:, :DH, :] / att_un[:, DH:DH + 1, :]
        # att: [8 heads, 64 dv, 1024 t] -> out cols [512g + 64h + dv]
        blk = att.transpose(2, 0, 1).reshape(T, KO)
        out[b, :, KO * g:KO * (g + 1)] = blk
    return out


def kernel(**inputs):
    data = np.asarray(inputs["data"], dtype=np.float32)
    W_in = np.asarray(inputs["W_in"], dtype=np.float32)
    W_q = np.asarray(inputs["W_q"], dtype=np.float32)
    W_k = np.asarray(inputs["W_k"], dtype=np.float32)
    W_v = np.asarray(inputs["W_v"], dtype=np.float32)

    in_maps = _prep_inputs(data, W_in, W_q, W_k, W_v)
    nc = _get_nc()
    res = run_bass_kernel_spmd(nc, in_maps, core_ids=list(range(B * G)))
    return _assemble(res.results)
